# revision 14
# baseline (speedup 1.0000x reference)
"""Diagonal complex SSM (LRU-style scan) on 8 trn2 NeuronCores.

y[t,p,k] = Re( C @ s[t,:,k] ) + (D @ x[t,:,k])
s[t,n,k] = A[n,k] * s[t-1,n,k] + (B @ x[t,:,k])[n]     (complex, diagonal)

Strategy: shard K=32 across 8 cores (4 lanes each; B/C/D replicated, no
collectives). Per core, linearize the complex scan by phase:
    A = r * e^{i theta};  s_t = e^{i theta t} * sh_t
    sh_t = r * sh_{t-1} + e^{-i theta t} * (B x_t)
so the recurrence becomes two independent REAL first-order scans per lane
(hardware tensor_tensor_scan, fp32 state), with elementwise rotations by
host-precomputed cos/sin(theta*t mod 2pi) tables. The rotate-out adds are
folded into the C matmul using negated stationaries:
    y = Cre*(c.sre - s.sim) - Cim*(c.sim + s.sre)
      = Cre@p1 + (-Cre)@p2 + (-Cim)@p3 + (-Cim)@p4
"""

import numpy as np

from concourse import bacc, mybir
from concourse.tile import TileContext
from concourse.bass_utils import run_bass_kernel_spmd

T, N, U, K, P = 4096, 256, 128, 32, 128
NCORES = 8
KL = K // NCORES          # k-lanes per core
TB = 512                  # timesteps per block (1 PSUM bank @ fp32)
NT = T // TB
F32 = mybir.dt.float32
F32R = mybir.dt.float32r  # full-rate PE matmul dtype (fp32 bits)

MM_F32R = True           # use float32r for all matmuls
MMDT = F32R if MM_F32R else F32

_CACHE = {}


def _build():
    nc = bacc.Bacc("TRN2", target_bir_lowering=False, debug=False,
                   num_devices=NCORES)

    xT_d = nc.dram_tensor("xT", [U, KL, T], MMDT, kind="ExternalInput")
    cos_d = nc.dram_tensor("cosT", [N, KL, T], F32, kind="ExternalInput")
    sin_d = nc.dram_tensor("sinT", [N, KL, T], F32, kind="ExternalInput")
    # r packed [p, h*KL + k] so each (h, k) lane-column is a [128,1] slice
    r_d = nc.dram_tensor("rdec", [128, 2 * KL], F32, kind="ExternalInput")
    Bre_d = nc.dram_tensor("BTre", [U, N], MMDT, kind="ExternalInput")
    Bim_d = nc.dram_tensor("BTim", [U, N], MMDT, kind="ExternalInput")
    C1_d = nc.dram_tensor("CT1", [128, N], MMDT, kind="ExternalInput")  # +Cre^T
    C2_d = nc.dram_tensor("CT2", [128, N], MMDT, kind="ExternalInput")  # -Cre^T
    C3_d = nc.dram_tensor("CT3", [128, N], MMDT, kind="ExternalInput")  # -Cim^T
    DT_d = nc.dram_tensor("DT", [U, P], MMDT, kind="ExternalInput")
    y_d = nc.dram_tensor("yT", [P, KL, T], F32, kind="ExternalOutput")

    mult = mybir.AluOpType.mult
    add = mybir.AluOpType.add

    def mmcast(ap):
        return ap

    with TileContext(nc) as tc:
        with (
            tc.tile_pool(name="const", bufs=1) as cpool,
            tc.tile_pool(name="xp", bufs=3) as xpool,
            tc.tile_pool(name="tab", bufs=3) as tabpool,
            tc.tile_pool(name="wk", bufs=3) as wk,
            tc.tile_pool(name="pr", bufs=3) as prpool,
            tc.tile_pool(name="sh", bufs=2) as shpool,
            tc.tile_pool(name="yo", bufs=3) as ypool,
            tc.tile_pool(name="ups", bufs=1, space="PSUM") as upsum,
            tc.tile_pool(name="yps", bufs=2, space="PSUM") as ypsum,
        ):
            Bre = cpool.tile([U, N], MMDT)
            nc.sync.dma_start(Bre[:], Bre_d[:])
            Bim = cpool.tile([U, N], MMDT)
            nc.sync.dma_start(Bim[:], Bim_d[:])
            C1 = cpool.tile([128, N], MMDT)
            nc.sync.dma_start(C1[:], C1_d[:])
            C2 = cpool.tile([128, N], MMDT)
            nc.sync.dma_start(C2[:], C2_d[:])
            C3 = cpool.tile([128, N], MMDT)
            nc.sync.dma_start(C3[:], C3_d[:])
            DT = cpool.tile([U, P], MMDT)
            nc.sync.dma_start(DT[:], DT_d[:])
            rsb = cpool.tile([128, 2 * KL], F32)
            nc.sync.dma_start(rsb[:], r_d[:])

            prev = {}
            for tb in range(NT):
                t0 = tb * TB
                for k in range(KL):
                    xt = xpool.tile([U, TB], MMDT, tag="x")
                    nc.sync.dma_start(xt[:], xT_d[:, k, t0:t0 + TB])

                    prods = []  # (stationary_tile, h, product_tile)
                    for h in (0, 1):
                        hs = slice(h * 128, (h + 1) * 128)
                        cos3 = tabpool.tile([128, 1, TB], F32, tag=f"cos{h}")
                        nc.sync.dma_start(cos3[:, 0, :], cos_d[hs, k, t0:t0 + TB])
                        sin3 = tabpool.tile([128, 1, TB], F32, tag=f"sin{h}")
                        nc.sync.dma_start(sin3[:, 0, :], sin_d[hs, k, t0:t0 + TB])
                        cost = cos3[:, 0, :]
                        sint = sin3[:, 0, :]

                        # u packed [im | re] in one 2-bank PSUM tile
                        u2 = upsum.tile([128, 2 * TB], F32, tag="u2")
                        nc.tensor.matmul(u2[:, 0:TB], mmcast(Bim[:, hs]),
                                         mmcast(xt[:]), start=True, stop=True)
                        nc.tensor.matmul(u2[:, TB:2 * TB], mmcast(Bre[:, hs]),
                                         mmcast(xt[:]), start=True, stop=True)
                        u3 = u2[:].rearrange("p (two tb) -> p two tb", two=2)

                        # rotate-in: uh = e^{-i th t} * u, paired FD=2*TB:
                        #   PP = [c|c] * [u_im|u_re],  QQ = [s|s] * [u_im|u_re]
                        #   uh_re = c*u_re + s*u_im = PP[re] + QQ[im]
                        #   uh_im = c*u_im - s*u_re = PP[im] - QQ[re]
                        # Port-aware: every DVE op keeps one operand in PSUM
                        # so the shared DVE/GpSimd SBUF port stays free.
                        cc = cos3[:].broadcast_to([128, 2, TB])
                        ss = sin3[:].broadcast_to([128, 2, TB])
                        PP = wk.tile([128, 2, TB], F32, tag="PP")
                        nc.vector.tensor_mul(PP[:], cc, u3)
                        QQ = upsum.tile([128, 2 * TB], F32, tag="QQ")
                        nc.vector.tensor_mul(
                            QQ[:].rearrange("p (two tb) -> p two tb", two=2),
                            ss, u3)
                        uh_re = upsum.tile([128, TB], F32, tag="uhre")
                        nc.vector.tensor_add(uh_re[:], PP[:, 1, :],
                                             QQ[:, 0:TB])
                        uh_im = upsum.tile([128, TB], F32, tag="uhim")
                        nc.vector.tensor_sub(uh_im[:], PP[:, 0, :],
                                             QQ[:, TB:2 * TB])

                        # hardware scans into a packed [im | re] tile.
                        # re on DVE (PSUM data1 keeps the shared SBUF port
                        # free); im on GpSimd.
                        ridx = h * KL + k
                        rb = rsb[:, ridx:ridx + 1].broadcast_to([128, TB])
                        sh2 = shpool.tile([128, 2 * TB], F32, tag=f"sh{k}{h}")
                        if tb == 0:
                            init_im, init_re = 0.0, 0.0
                        else:
                            pv = prev[(k, h)]
                            init_im = pv[:, TB - 1:TB]
                            init_re = pv[:, 2 * TB - 1:2 * TB]
                        nc.vector.tensor_tensor_scan(
                            sh2[:, 0:TB], rb, uh_im[:], init_im, mult, add)
                        nc.vector.tensor_tensor_scan(
                            sh2[:, TB:2 * TB], rb, uh_re[:], init_re,
                            mult, add)
                        prev[(k, h)] = sh2

                        # rotate-out products on GpSimd (concurrent with
                        # DVE thanks to PSUM-operand port discipline)
                        sh_im = sh2[:, 0:TB]
                        sh_re = sh2[:, TB:2 * TB]
                        p1 = prpool.tile([128, TB], MMDT, tag=f"p1{h}")
                        nc.gpsimd.tensor_mul(p1[:], cost[:], sh_re)
                        p2 = prpool.tile([128, TB], MMDT, tag=f"p2{h}")
                        nc.gpsimd.tensor_mul(p2[:], sint[:], sh_im)
                        p3 = prpool.tile([128, TB], MMDT, tag=f"p3{h}")
                        nc.gpsimd.tensor_mul(p3[:], cost[:], sh_im)
                        p4 = prpool.tile([128, TB], MMDT, tag=f"p4{h}")
                        nc.gpsimd.tensor_mul(p4[:], sint[:], sh_re)
                        prods += [(C1, h, p1[:]), (C2, h, p2[:]),
                                  (C3, h, p3[:]), (C3, h, p4[:])]

                    y_ps = ypsum.tile([P, TB], F32, tag="y")
                    nmm = len(prods) + 1
                    for i, (cst, h, pt) in enumerate(prods):
                        hs = slice(h * 128, (h + 1) * 128)
                        nc.tensor.matmul(y_ps[:], mmcast(cst[:, hs]),
                                         mmcast(pt),
                                         start=(i == 0), stop=False)
                    nc.tensor.matmul(y_ps[:], mmcast(DT[:]), mmcast(xt[:]),
                                     start=False, stop=True)

                    y_sb = ypool.tile([P, TB], F32, tag="ysb")
                    nc.scalar.copy(y_sb[:], y_ps[:])
                    nc.sync.dma_start(y_d[:, k, t0:t0 + TB], y_sb[:])

    nc.compile()
    return nc


def _host_prep(input_sequence, A_re, A_im, B_re, B_im, C_re, C_im, D):
    """Build the per-core input maps (numpy only)."""
    # Accept numpy or jax arrays.
    input_sequence = np.asarray(input_sequence, dtype=np.float32)
    A_re = np.asarray(A_re, dtype=np.float32)
    A_im = np.asarray(A_im, dtype=np.float32)
    B_re = np.asarray(B_re, dtype=np.float32)
    B_im = np.asarray(B_im, dtype=np.float32)
    C_re = np.asarray(C_re, dtype=np.float32)
    C_im = np.asarray(C_im, dtype=np.float32)
    D = np.asarray(D, dtype=np.float32)
    x = np.ascontiguousarray(input_sequence, dtype=np.float32)
    th = np.arctan2(A_im.astype(np.float64), A_re.astype(np.float64))  # (N,K)
    r = np.hypot(A_re.astype(np.float64), A_im.astype(np.float64))    # (N,K)

    t = np.arange(T, dtype=np.float64)
    # angle = theta * t  (mod 2pi), computed in fp64 then reduced
    ang = (th[:, :, None] * t[None, None, :]) % (2 * np.pi)  # (N, K, T)
    cosT = np.cos(ang).astype(np.float32)
    sinT = np.sin(ang).astype(np.float32)

    BTre = np.ascontiguousarray(B_re.T, dtype=np.float32)   # (U, N)
    BTim = np.ascontiguousarray(B_im.T, dtype=np.float32)
    CT1 = np.concatenate([C_re[:, :128].T, C_re[:, 128:].T], axis=1)
    CT2 = -CT1
    CT3 = np.concatenate([-C_im[:, :128].T, -C_im[:, 128:].T], axis=1)
    CT1 = np.ascontiguousarray(CT1, dtype=np.float32)       # (128, N)
    CT2 = np.ascontiguousarray(CT2, dtype=np.float32)
    CT3 = np.ascontiguousarray(CT3, dtype=np.float32)
    DT = np.ascontiguousarray(D.T, dtype=np.float32)        # (U, P)

    in_maps = []
    for c in range(NCORES):
        ks = slice(c * KL, (c + 1) * KL)
        xT = np.ascontiguousarray(x[:, :, ks].transpose(1, 2, 0))  # (U,KL,T)
        cosc = np.ascontiguousarray(cosT[:, ks, :])                # (N,KL,T)
        sinc = np.ascontiguousarray(sinT[:, ks, :])
        rc = r[:, ks].astype(np.float32)                           # (N,KL)
        # pack as [p, h*KL + k]
        rpk = np.concatenate([rc[:128, :], rc[128:, :]], axis=1)   # (128,2KL)
        rpk = np.ascontiguousarray(rpk, dtype=np.float32)
        in_maps.append(dict(xT=xT, cosT=cosc, sinT=sinc, rdec=rpk,
                            BTre=BTre, BTim=BTim, CT1=CT1, CT2=CT2,
                            CT3=CT3, DT=DT))
    return in_maps


def _get_nc():
    if "nc" not in _CACHE:
        _CACHE["nc"] = _build()
    return _CACHE["nc"]


def kernel(input_sequence, A_re, A_im, B_re, B_im, C_re, C_im, D,
           trace=False):
    nc = _get_nc()
    in_maps = _host_prep(input_sequence, A_re, A_im, B_re, B_im, C_re,
                         C_im, D)
    res = run_bass_kernel_spmd(nc, in_maps, core_ids=list(range(NCORES)),
                               trace=trace)
    out = np.empty((T, P, K), dtype=np.float32)
    for c in range(NCORES):
        yT = res.results[c]["yT"]                    # (P, KL, T)
        out[:, :, c * KL:(c + 1) * KL] = yT.transpose(2, 0, 1)
    if trace:
        _CACHE["exec_time_ns"] = res.exec_time_ns
    return out


# revision 15
# speedup vs baseline: 1.2016x; 1.2016x over previous
"""Diagonal complex SSM (LRU-style scan) on 8 trn2 NeuronCores.

y[t,p,k] = Re( C @ s[t,:,k] ) + (D @ x[t,:,k])
s[t,n,k] = A[n,k] * s[t-1,n,k] + (B @ x[t,:,k])[n]     (complex, diagonal)

Strategy: shard K=32 across 8 cores (4 lanes each; B/C/D replicated, no
collectives). Per core, linearize the complex scan by phase:
    A = r * e^{i theta};  s_t = e^{i theta t} * sh_t
    sh_t = r * sh_{t-1} + e^{-i theta t} * (B x_t)
so the recurrence becomes two independent REAL first-order scans per lane
(hardware tensor_tensor_scan, fp32 state), with elementwise rotations by
host-precomputed cos/sin(theta*t mod 2pi) tables. The rotate-out adds are
folded into the C matmul using negated stationaries:
    y = Cre*(c.sre - s.sim) - Cim*(c.sim + s.sre)
      = Cre@p1 + (-Cre)@p2 + (-Cim)@p3 + (-Cim)@p4
"""

import numpy as np

from concourse import bacc, mybir
from concourse.tile import TileContext
from concourse.bass_utils import run_bass_kernel_spmd

T, N, U, K, P = 4096, 256, 128, 32, 128
NCORES = 8
KL = K // NCORES          # k-lanes per core
TB = 512                  # timesteps per block (1 PSUM bank @ fp32)
NT = T // TB
F32 = mybir.dt.float32
F32R = mybir.dt.float32r  # full-rate PE matmul dtype (fp32 bits)

MM_F32R = True           # use float32r for all matmuls
MMDT = F32R if MM_F32R else F32

_CACHE = {}


def _build():
    nc = bacc.Bacc("TRN2", target_bir_lowering=False, debug=False,
                   num_devices=NCORES)

    xT_d = nc.dram_tensor("xT", [U, KL, T], MMDT, kind="ExternalInput")
    cos_d = nc.dram_tensor("cosT", [N, KL, T], F32, kind="ExternalInput")
    sin_d = nc.dram_tensor("sinT", [N, KL, T], F32, kind="ExternalInput")
    # r packed [p, h*KL + k] so each (h, k) lane-column is a [128,1] slice
    r_d = nc.dram_tensor("rdec", [128, 2 * KL], F32, kind="ExternalInput")
    Bre_d = nc.dram_tensor("BTre", [U, N], MMDT, kind="ExternalInput")
    Bim_d = nc.dram_tensor("BTim", [U, N], MMDT, kind="ExternalInput")
    C1_d = nc.dram_tensor("CT1", [128, N], MMDT, kind="ExternalInput")  # +Cre^T
    C2_d = nc.dram_tensor("CT2", [128, N], MMDT, kind="ExternalInput")  # -Cre^T
    C3_d = nc.dram_tensor("CT3", [128, N], MMDT, kind="ExternalInput")  # -Cim^T
    DT_d = nc.dram_tensor("DT", [U, P], MMDT, kind="ExternalInput")
    y_d = nc.dram_tensor("yT", [P, KL, T], F32, kind="ExternalOutput")

    mult = mybir.AluOpType.mult
    add = mybir.AluOpType.add

    def mmcast(ap):
        return ap

    with TileContext(nc) as tc:
        with (
            tc.tile_pool(name="const", bufs=1) as cpool,
            tc.tile_pool(name="xp", bufs=3) as xpool,
            tc.tile_pool(name="tab", bufs=3) as tabpool,
            tc.tile_pool(name="wk", bufs=3) as wk,
            tc.tile_pool(name="pr", bufs=3) as prpool,
            tc.tile_pool(name="sh", bufs=2) as shpool,
            tc.tile_pool(name="yo", bufs=3) as ypool,
            tc.tile_pool(name="ups", bufs=1, space="PSUM") as upsum,
            tc.tile_pool(name="yps", bufs=2, space="PSUM") as ypsum,
        ):
            Bre = cpool.tile([U, N], MMDT)
            nc.sync.dma_start(Bre[:], Bre_d[:])
            Bim = cpool.tile([U, N], MMDT)
            nc.sync.dma_start(Bim[:], Bim_d[:])
            C1 = cpool.tile([128, N], MMDT)
            nc.sync.dma_start(C1[:], C1_d[:])
            C2 = cpool.tile([128, N], MMDT)
            nc.sync.dma_start(C2[:], C2_d[:])
            C3 = cpool.tile([128, N], MMDT)
            nc.sync.dma_start(C3[:], C3_d[:])
            DT = cpool.tile([U, P], MMDT)
            nc.sync.dma_start(DT[:], DT_d[:])
            rsb = cpool.tile([128, 2 * KL], F32)
            nc.sync.dma_start(rsb[:], r_d[:])

            prev = {}
            for tb in range(NT):
                t0 = tb * TB
                for k in range(KL):
                    xt = xpool.tile([U, TB], MMDT, tag="x")
                    nc.sync.dma_start(xt[:], xT_d[:, k, t0:t0 + TB])

                    prods = []  # (stationary_tile, h, product_tile)
                    for h in (0, 1):
                        hs = slice(h * 128, (h + 1) * 128)
                        cos3 = tabpool.tile([128, 1, TB], F32, tag=f"cos{h}")
                        nc.sync.dma_start(cos3[:, 0, :], cos_d[hs, k, t0:t0 + TB])
                        sin3 = tabpool.tile([128, 1, TB], F32, tag=f"sin{h}")
                        nc.sync.dma_start(sin3[:, 0, :], sin_d[hs, k, t0:t0 + TB])
                        cost = cos3[:, 0, :]
                        sint = sin3[:, 0, :]

                        u_re = upsum.tile([128, TB], F32, tag="ure")
                        u_im = upsum.tile([128, TB], F32, tag="uim")
                        nc.tensor.matmul(u_re[:], mmcast(Bre[:, hs]),
                                         mmcast(xt[:]), start=True, stop=True)
                        nc.tensor.matmul(u_im[:], mmcast(Bim[:, hs]),
                                         mmcast(xt[:]), start=True, stop=True)

                        # rotate-in: uh = e^{-i th t} * u.  Port-aware: every
                        # DVE op below keeps one operand in PSUM so the
                        # DVE/GpSimd shared SBUF port stays free for the
                        # GpSimd rotate-out products.
                        t1 = wk.tile([128, TB], F32, tag="t1")
                        nc.vector.tensor_mul(t1[:], cost[:], u_re[:])
                        t2 = upsum.tile([128, TB], F32, tag="t2")
                        nc.vector.tensor_mul(t2[:], sint[:], u_im[:])
                        uh_re = upsum.tile([128, TB], F32, tag="uhre")
                        nc.vector.tensor_add(uh_re[:], t1[:], t2[:])
                        t3 = wk.tile([128, TB], F32, tag="t3")
                        nc.vector.tensor_mul(t3[:], cost[:], u_im[:])
                        t4 = upsum.tile([128, TB], F32, tag="t4")
                        nc.vector.tensor_mul(t4[:], sint[:], u_re[:])
                        uh_im = upsum.tile([128, TB], F32, tag="uhim")
                        nc.vector.tensor_sub(uh_im[:], t3[:], t4[:])

                        # hardware scans into a packed [im | re] tile.
                        # re on DVE (PSUM data1 keeps the shared SBUF port
                        # free); im on GpSimd.
                        ridx = h * KL + k
                        rb = rsb[:, ridx:ridx + 1].broadcast_to([128, TB])
                        sh2 = shpool.tile([128, 2 * TB], F32, tag=f"sh{k}{h}")
                        if tb == 0:
                            init_im, init_re = 0.0, 0.0
                        else:
                            pv = prev[(k, h)]
                            init_im = pv[:, TB - 1:TB]
                            init_re = pv[:, 2 * TB - 1:2 * TB]
                        nc.vector.tensor_tensor_scan(
                            sh2[:, 0:TB], rb, uh_im[:], init_im, mult, add)
                        nc.vector.tensor_tensor_scan(
                            sh2[:, TB:2 * TB], rb, uh_re[:], init_re,
                            mult, add)
                        prev[(k, h)] = sh2

                        # rotate-out products on GpSimd (concurrent with
                        # DVE thanks to PSUM-operand port discipline)
                        sh_im = sh2[:, 0:TB]
                        sh_re = sh2[:, TB:2 * TB]
                        p1 = prpool.tile([128, TB], MMDT, tag=f"p1{h}")
                        nc.gpsimd.tensor_mul(p1[:], cost[:], sh_re)
                        p2 = prpool.tile([128, TB], MMDT, tag=f"p2{h}")
                        nc.gpsimd.tensor_mul(p2[:], sint[:], sh_im)
                        p3 = prpool.tile([128, TB], MMDT, tag=f"p3{h}")
                        nc.gpsimd.tensor_mul(p3[:], cost[:], sh_im)
                        p4 = prpool.tile([128, TB], MMDT, tag=f"p4{h}")
                        nc.gpsimd.tensor_mul(p4[:], sint[:], sh_re)
                        prods += [(C1, h, p1[:]), (C2, h, p2[:]),
                                  (C3, h, p3[:]), (C3, h, p4[:])]

                    y_ps = ypsum.tile([P, TB], F32, tag="y")
                    nmm = len(prods) + 1
                    for i, (cst, h, pt) in enumerate(prods):
                        hs = slice(h * 128, (h + 1) * 128)
                        nc.tensor.matmul(y_ps[:], mmcast(cst[:, hs]),
                                         mmcast(pt),
                                         start=(i == 0), stop=False)
                    nc.tensor.matmul(y_ps[:], mmcast(DT[:]), mmcast(xt[:]),
                                     start=False, stop=True)

                    y_sb = ypool.tile([P, TB], F32, tag="ysb")
                    nc.scalar.copy(y_sb[:], y_ps[:])
                    nc.sync.dma_start(y_d[:, k, t0:t0 + TB], y_sb[:])

    nc.compile()
    return nc


def _host_prep(input_sequence, A_re, A_im, B_re, B_im, C_re, C_im, D):
    """Build the per-core input maps (numpy only)."""
    # Accept numpy or jax arrays.
    input_sequence = np.asarray(input_sequence, dtype=np.float32)
    A_re = np.asarray(A_re, dtype=np.float32)
    A_im = np.asarray(A_im, dtype=np.float32)
    B_re = np.asarray(B_re, dtype=np.float32)
    B_im = np.asarray(B_im, dtype=np.float32)
    C_re = np.asarray(C_re, dtype=np.float32)
    C_im = np.asarray(C_im, dtype=np.float32)
    D = np.asarray(D, dtype=np.float32)
    x = np.ascontiguousarray(input_sequence, dtype=np.float32)
    th = np.arctan2(A_im.astype(np.float64), A_re.astype(np.float64))  # (N,K)
    r = np.hypot(A_re.astype(np.float64), A_im.astype(np.float64))    # (N,K)

    t = np.arange(T, dtype=np.float64)
    # angle = theta * t  (mod 2pi), computed in fp64 then reduced
    ang = (th[:, :, None] * t[None, None, :]) % (2 * np.pi)  # (N, K, T)
    cosT = np.cos(ang).astype(np.float32)
    sinT = np.sin(ang).astype(np.float32)

    BTre = np.ascontiguousarray(B_re.T, dtype=np.float32)   # (U, N)
    BTim = np.ascontiguousarray(B_im.T, dtype=np.float32)
    CT1 = np.concatenate([C_re[:, :128].T, C_re[:, 128:].T], axis=1)
    CT2 = -CT1
    CT3 = np.concatenate([-C_im[:, :128].T, -C_im[:, 128:].T], axis=1)
    CT1 = np.ascontiguousarray(CT1, dtype=np.float32)       # (128, N)
    CT2 = np.ascontiguousarray(CT2, dtype=np.float32)
    CT3 = np.ascontiguousarray(CT3, dtype=np.float32)
    DT = np.ascontiguousarray(D.T, dtype=np.float32)        # (U, P)

    in_maps = []
    for c in range(NCORES):
        ks = slice(c * KL, (c + 1) * KL)
        xT = np.ascontiguousarray(x[:, :, ks].transpose(1, 2, 0))  # (U,KL,T)
        cosc = np.ascontiguousarray(cosT[:, ks, :])                # (N,KL,T)
        sinc = np.ascontiguousarray(sinT[:, ks, :])
        rc = r[:, ks].astype(np.float32)                           # (N,KL)
        # pack as [p, h*KL + k]
        rpk = np.concatenate([rc[:128, :], rc[128:, :]], axis=1)   # (128,2KL)
        rpk = np.ascontiguousarray(rpk, dtype=np.float32)
        in_maps.append(dict(xT=xT, cosT=cosc, sinT=sinc, rdec=rpk,
                            BTre=BTre, BTim=BTim, CT1=CT1, CT2=CT2,
                            CT3=CT3, DT=DT))
    return in_maps


def _get_nc():
    if "nc" not in _CACHE:
        _CACHE["nc"] = _build()
    return _CACHE["nc"]


def kernel(input_sequence, A_re, A_im, B_re, B_im, C_re, C_im, D,
           trace=False):
    nc = _get_nc()
    in_maps = _host_prep(input_sequence, A_re, A_im, B_re, B_im, C_re,
                         C_im, D)
    res = run_bass_kernel_spmd(nc, in_maps, core_ids=list(range(NCORES)),
                               trace=trace)
    out = np.empty((T, P, K), dtype=np.float32)
    for c in range(NCORES):
        yT = res.results[c]["yT"]                    # (P, KL, T)
        out[:, :, c * KL:(c + 1) * KL] = yT.transpose(2, 0, 1)
    if trace:
        _CACHE["exec_time_ns"] = res.exec_time_ns
    return out
